# revision 24
# baseline (speedup 1.0000x reference)
"""Trainium2 Bass kernel for nn_CombinedLoss (LCCNet CombinedLoss).

Per sample, displacement d = A3 @ q + a4. Column-pivoted QR gives
    err^2 = (s1*(u1 + a*u2 + b*u3) + b1)^2 + (s2*(u2 + g*u3) + b2)^2 + b3^2
with bounded |a|,|b|,|g| <= 1. Host prescales u1' = s1*u1 + b1 so the
first square needs no scale/bias (t1 = f1'^2, a plain DVE multiply).

Device mapping (8 cores, data-parallel; a core's 4 samples share the
partition dim: p = 32*sample + slot, each slot FREE=6240 points):
  - Pool: chunked fp32->fp16 cast DMAs (the ~25us read roofline) plus
    the three big s12 = t1+t2 adds.
  - PE: c2 = g.*u3 + u2 via two diagonal-matmul accumulates per 390-col
    PSUM bank slot (slot table avoids bank wraps); ACT reads PSUM.
  - DVE: c1' = (b*s1).*u3 + u1', f1' = (a*s1).*u2 + c1' (STT), the small
    s12 adds, and t1 = f1'*f1' for the two middle chunks.
  - ACT: t2 = Square(s2.*c2_psum + b2), t1 squares for the small chunks,
    err = Sqrt(s12 + b3sq) with accum_out (sqrts merged pairwise at the
    edges to cut accumulator reads).
  - Chunks [390,390,1560,1560,1560,390,390]: fast fill, short tail.
  - Host: tails + pose loss in float64.
"""

import numpy as np

B = 32
N = 200000
NCORES = 8
SPC = B // NCORES
NPART = 128
SLOTS = 32
FREE = 6240
NDEV = SLOTS * FREE        # 199680 points per sample on device
NCONST = 8

# DMA: per-coord span transfers (one coord x 1560 cols each -> 6240B
# DRAM lines, full stream rate; narrower lines cost ~20% bandwidth).
# Per span the order is u3, u1, u2 so c1 can start before u2 lands.
SPAN_W = 1560
NSPAN = FREE // SPAN_W      # 4
CCH = [780, 1560, 1560, 1560, 390, 390]
PEW = 390
NPAIR = FREE // PEW        # 16
NSLOT = 8
SLOT_STRIDE = 512          # fp32 elems per 2KB PSUM bank

# PSUM bank slot per PE pair (chunk's pairs stay contiguous, no wraps)
PAIR_SLOT = [6, 7, 0, 1, 2, 3, 4, 5, 6, 7, 0, 1, 2, 3, 4, 5]

_CACHED_NC = None


def _offsets(widths):
    offs, o = [], 0
    for w in widths:
        offs.append(o)
        o += w
    return offs


C_OFF = _offsets(CCH)
# spans covering each compute chunk; sem_u index = 3*span + {0:u3,1:u1,2:u2}
C_SPAN = [[s for s in range(NSPAN)
           if s * SPAN_W < o + w and (s + 1) * SPAN_W > o]
          for o, w in zip(C_OFF, CCH)]
PAIR_SPAN = [(p * PEW) // SPAN_W for p in range(NPAIR)]
C_PAIR = [(o // PEW, w // PEW) for o, w in zip(C_OFF, CCH)]

DVE_T1 = (3,)              # chunks whose t1 square runs on DVE
# merged sqrt ops: (chunks, offset, width, acc column). GPSIMD compute
# shares the DMA path, so Pool stays DMA-only while the stream runs.
SQ_OPS = [((0,), 0, 780, 0), ((1,), 780, 1560, 1), ((2,), 2340, 1560, 2),
          ((3,), 3900, 1560, 3), ((4, 5), 5460, 780, 4)]
NACC = len(SQ_OPS)


def _quat_to_rot(q):
    q = q / np.linalg.norm(q)
    w, x, y, z = q
    return np.array([
        [1 - 2*y*y - 2*z*z, 2*x*y - 2*z*w,     2*x*z + 2*y*w],
        [2*x*y + 2*z*w,     1 - 2*x*x - 2*z*z, 2*y*z - 2*x*w],
        [2*x*z - 2*y*w,     2*y*z + 2*x*w,     1 - 2*x*x - 2*y*y],
    ])


def _pivoted_qr(A3):
    """Column-pivoted QR (float64): A3[:, piv] = Q @ R, |R[i,j]| <= |R[i,i]|."""
    cols = {c: A3[:, c].astype(np.float64).copy() for c in range(3)}
    coeff = {c: np.zeros(3) for c in range(3)}
    remaining = [0, 1, 2]
    piv = []
    Q = np.zeros((3, 3))
    for i in range(3):
        cbest = max(remaining, key=lambda c: float(np.dot(cols[c], cols[c])))
        remaining.remove(cbest)
        piv.append(cbest)
        v = cols[cbest]
        nrm = np.sqrt(np.dot(v, v))
        if nrm < 1e-300:
            for basis in np.eye(3):
                w = basis - Q[:, :i] @ (Q[:, :i].T @ basis)
                if np.dot(w, w) > 1e-12:
                    v = w
                    break
            nrm = np.sqrt(np.dot(v, v))
        q = v / nrm
        Q[:, i] = q
        for c in [cbest] + remaining:
            proj = float(np.dot(q, cols[c]))
            coeff[c][i] = proj
            cols[c] = cols[c] - proj * q
    R = np.stack([coeff[c] for c in piv], axis=1)
    return Q, R, piv


def _per_sample_host(tt, tr, te, re_):
    """(A3, a4, piv, consts). consts = [a*s1, b*s1, g, s1, b1, s2, b2, b3sq]
    where s1=r11, b1=b4[0] also get folded into the u1 stream on the host."""
    R_t = _quat_to_rot(tr.astype(np.float64))
    R_p = _quat_to_rot(re_.astype(np.float64))
    A3 = R_p.T @ R_t - np.eye(3)
    a4 = R_p.T @ (tt.astype(np.float64) - te.astype(np.float64))
    Q, R, piv = _pivoted_qr(A3)
    b4 = Q.T @ a4
    r11, r12, r13 = R[0, 0], R[0, 1], R[0, 2]
    r22, r23 = R[1, 1], R[1, 2]
    alpha = r12 / r11 if abs(r11) > 1e-30 else 0.0
    beta = r13 / r11 if abs(r11) > 1e-30 else 0.0
    gamma = r23 / r22 if abs(r22) > 1e-30 else 0.0
    consts = np.array([alpha * r11, beta * r11, gamma, r11, b4[0], r22,
                       b4[1], b4[2] ** 2])
    return A3, a4, piv, consts


def _build_nc():
    import concourse.bass as bass
    from concourse import mybir
    from contextlib import ExitStack

    f16, f32 = mybir.dt.float16, mybir.dt.float32
    Alu = mybir.AluOpType
    Act = mybir.ActivationFunctionType

    nc = bass.Bass("TRN2", target_bir_lowering=False, debug=False,
                   num_devices=NCORES)
    pc = nc.dram_tensor("pc", [3, NPART, FREE], f32, kind="ExternalInput").ap()
    cst = nc.dram_tensor("cst", [NPART, NCONST], f32,
                         kind="ExternalInput").ap()
    wts = nc.dram_tensor("wts", [NPART, 2 * NPART], f16,
                         kind="ExternalInput").ap()
    acc_out = nc.dram_tensor("acc", [NPART, NACC], f32,
                             kind="ExternalOutput").ap()

    NCH = len(CCH)

    # ---- ACT program order and tick numbering (1 inc per ACT op) ----
    act_order = [("t2", 0), ("t1", 0), ("t2", 1), ("t1", 1), ("sq", 0),
                 ("t2", 2), ("t1", 2), ("sq", 1), ("t2", 3), ("sq", 2),
                 ("t2", 4), ("t1", 4), ("t2", 5), ("t1", 5), ("sq", 3),
                 ("sq", 4)]
    ACT_TICK = {op: i + 1 for i, op in enumerate(act_order)}
    ACT_TOTAL = len(act_order)

    # ---- DVE sem_dve tick numbering ----
    dve_order = [("f1", 0), ("f1", 1), ("f1", 2), ("f1", 3), ("t1d", 3),
                 ("f1", 4), ("f1", 5)]
    DVE_TICK = {op: i + 1 for i, op in enumerate(dve_order)}
    # s12 runs on DVE in program order; sem_s12 count per chunk
    s12_order = [0, 1, 2, 3, 4, 5]
    S12_TICK = {c: i + 1 for i, c in enumerate(s12_order)}

    # PE slot-reuse waits: pair -> required sem_act tick (slot written 8
    # pairs earlier must have been consumed by that chunk's t2)
    PE_WAIT = {8: ACT_TICK[("t2", 0)], 10: ACT_TICK[("t2", 1)],
               14: ACT_TICK[("t2", 2)]}

    with ExitStack() as ctx:
        E = ctx.enter_context
        u = E(nc.sbuf_tensor("u", [NPART, 3 * FREE], f16))
        ct = E(nc.sbuf_tensor("ct", [NPART, NCONST], f32))
        wt = E(nc.sbuf_tensor("wt", [NPART, 2 * NPART], f16))
        c1b = E(nc.sbuf_tensor("c1b", [NPART, FREE], f16))
        f1b = E(nc.sbuf_tensor("f1b", [NPART, FREE], f16))
        t1b = E(nc.sbuf_tensor("t1b", [NPART, FREE], f16))
        t2b = E(nc.sbuf_tensor("t2b", [NPART, FREE], f16))
        s12b = E(nc.sbuf_tensor("s12b", [NPART, FREE], f16))
        esb = E(nc.sbuf_tensor("esb", [NPART, 1560], f16))
        acc = E(nc.sbuf_tensor("acc_sb", [NPART, NACC], f32))
        ps = E(nc.psum_tensor("ps", [NPART, NSLOT * SLOT_STRIDE], f32))

        sem_c = E(nc.semaphore("sem_c"))
        sem_u = [E(nc.semaphore(f"sem_u{d}")) for d in range(3 * NSPAN)]
        sem_pe = E(nc.semaphore("sem_pe"))
        sem_dve = E(nc.semaphore("sem_dve"))
        sem_act = E(nc.semaphore("sem_act"))
        sem_s12 = E(nc.semaphore("sem_s12"))
        sem_out = E(nc.semaphore("sem_out"))
        block = E(nc.Block(no_gpsimd_drain=True))

        def ccol(i):
            return ct[:, i:i + 1]

        def useg(coord, o, w):
            return u[:, coord * FREE + o:coord * FREE + o + w]

        wg = wt[:, 0:NPART]
        wi = wt[:, NPART:2 * NPART]

        @block.gpsimd
        def _(g):
            def span_dma(s, coord, slot):
                o = s * SPAN_W
                g.dma_start(
                    useg(coord, o, SPAN_W),
                    pc[coord, :, o:o + SPAN_W],
                ).then_inc(sem_u[3 * s + slot], 16)

            g.dma_start(ct[:], cst).then_inc(sem_c, 16)
            g.dma_start(wt[:], wts).then_inc(sem_c, 16)
            for s in range(NSPAN):
                span_dma(s, 2, 0)   # u3 first: c1 and the PE both need it
                span_dma(s, 0, 1)   # u1 (prescaled): c1
                span_dma(s, 1, 2)   # u2: f1 and the PE second matmul

        @block.tensor
        def _(t):
            t.wait_ge(sem_c, 32)
            for _i in range(6):
                t.matmul(ps[:, 7 * SLOT_STRIDE:7 * SLOT_STRIDE + PEW],
                         wi, useg(2, 0, PEW), start=True, stop=True)
            last_s = -1
            for p in range(NPAIR):
                sp_ = PAIR_SPAN[p]
                if sp_ != last_s:
                    t.wait_ge(sem_u[3 * sp_ + 0], 16)   # u3
                    t.wait_ge(sem_u[3 * sp_ + 2], 16)   # u2
                    last_s = sp_
                if p in PE_WAIT:
                    t.wait_ge(sem_act, PE_WAIT[p])
                s0 = PAIR_SLOT[p] * SLOT_STRIDE
                slot = ps[:, s0:s0 + PEW]
                o = p * PEW
                t.matmul(slot, wg, useg(2, o, PEW), start=True, stop=False)
                t.matmul(slot, wi, useg(1, o, PEW), start=False, stop=True) \
                    .then_inc(sem_pe, 1)

        @block.vector
        def _(v):
            v.wait_ge(sem_c, 16)

            def K(c):
                o, w = C_OFF[c], CCH[c]
                for sp_ in C_SPAN[c]:
                    v.wait_ge(sem_u[3 * sp_ + 0], 16)   # u3
                    v.wait_ge(sem_u[3 * sp_ + 1], 16)   # u1
                v.scalar_tensor_tensor(c1b[:, o:o + w], useg(2, o, w),
                                       ccol(1), useg(0, o, w),
                                       Alu.mult, Alu.add)
                v.drain()  # f1 reads c1 on the same engine
                for sp_ in C_SPAN[c]:
                    v.wait_ge(sem_u[3 * sp_ + 2], 16)   # u2
                v.scalar_tensor_tensor(f1b[:, o:o + w], useg(1, o, w),
                                       ccol(0), c1b[:, o:o + w],
                                       Alu.mult, Alu.add) \
                    .then_inc(sem_dve, 1)

            def t1d(c):
                o, w = C_OFF[c], CCH[c]
                v.drain()  # reads f1 written by the previous STT
                v.tensor_tensor(t1b[:, o:o + w], f1b[:, o:o + w],
                                f1b[:, o:o + w], Alu.mult) \
                    .then_inc(sem_dve, 1)

            def S(c):
                o, w = C_OFF[c], CCH[c]
                if c in DVE_T1:
                    # t1 is local; only t2 comes from ACT
                    v.wait_ge(sem_act, ACT_TICK[("t2", c)])
                else:
                    v.wait_ge(sem_act, ACT_TICK[("t1", c)])
                v.tensor_tensor(s12b[:, o:o + w], t1b[:, o:o + w],
                                t2b[:, o:o + w], Alu.add).then_inc(sem_s12, 1)

            K(0)
            K(1)
            S(0)
            K(2)
            S(1)
            K(3)
            t1d(3)
            S(2)
            S(3)
            K(4)
            K(5)
            S(4)
            S(5)

        @block.scalar
        def _(s):
            # prime the Square/Sqrt table off the critical path
            s.activation(esb[:, 0:1], esb[:, 1:2], Act.Square)
            s.wait_ge(sem_c, 16)
            sq_done = [0]

            def t2(c):
                o, w = C_OFF[c], CCH[c]
                p0, np_ = C_PAIR[c]
                s.wait_ge(sem_pe, p0 + np_)
                s0 = PAIR_SLOT[p0]
                pin = ps[:].rearrange("p (k q) -> p k q", k=NSLOT)[
                    :, s0:s0 + np_, 0:PEW]
                s.activation(
                    t2b[:, o:o + w].rearrange("p (k f) -> p k f", k=np_),
                    pin, Act.Square, bias=ccol(6), scale=ccol(5)) \
                    .then_inc(sem_act, 1)

            def t1(c):
                o, w = C_OFF[c], CCH[c]
                s.wait_ge(sem_dve, DVE_TICK[("f1", c)])
                s.activation(t1b[:, o:o + w], f1b[:, o:o + w], Act.Square,
                             bias=ccol(4), scale=ccol(3)) \
                    .then_inc(sem_act, 1)

            def sq(i):
                chunks, o, w, col = SQ_OPS[i]
                s.wait_ge(sem_s12, S12_TICK[chunks[-1]])
                s.activation(esb[:, 0:w], s12b[:, o:o + w], Act.Sqrt,
                             bias=ccol(7), accum_out=acc[:, col:col + 1]) \
                    .then_inc(sem_act, 1)

            emit = {"t2": t2, "t1": t1}
            for kind, cc in act_order:
                if kind == "sq":
                    sq(cc)
                else:
                    emit[kind](cc)

        @block.sync
        def _(sp):
            sp.wait_ge(sem_act, ACT_TOTAL)
            sp.dma_start(acc_out, acc[:]).then_inc(sem_out, 16)
            sp.wait_ge(sem_out, 16)

    return nc


def _get_nc():
    global _CACHED_NC
    if _CACHED_NC is None:
        _CACHED_NC = _build_nc()
    return _CACHED_NC


def _kernel_impl(point_clouds, target_transl, target_rot, transl_err, rot_err,
                 trace=False):
    from concourse.bass_utils import run_bass_kernel_spmd

    pc = np.asarray(point_clouds)
    tt = np.asarray(target_transl, np.float64)
    tr = np.asarray(target_rot, np.float64)
    te = np.asarray(transl_err, np.float64)
    re_ = np.asarray(rot_err, np.float64)

    # ---- pose loss (host, float64) ----
    d = np.abs(te - tt)
    loss_transl = np.where(d < 1.0, 0.5 * d * d, d - 0.5).sum(axis=1).mean()

    rinv = tr * np.array([1.0, -1.0, -1.0, -1.0])
    q = re_
    w = q[:, 0]*rinv[:, 0] - q[:, 1]*rinv[:, 1] - q[:, 2]*rinv[:, 2] - q[:, 3]*rinv[:, 3]
    x = q[:, 0]*rinv[:, 1] + q[:, 1]*rinv[:, 0] + q[:, 2]*rinv[:, 3] - q[:, 3]*rinv[:, 2]
    y = q[:, 0]*rinv[:, 2] - q[:, 1]*rinv[:, 3] + q[:, 2]*rinv[:, 0] + q[:, 3]*rinv[:, 1]
    z = q[:, 0]*rinv[:, 3] + q[:, 1]*rinv[:, 2] - q[:, 2]*rinv[:, 1] + q[:, 3]*rinv[:, 0]
    angle = 2.0 * np.arctan2(np.sqrt(x*x + y*y + z*z), np.abs(w))
    loss_rot = (180.0 * angle / np.pi).mean()
    pose_loss = loss_transl + loss_rot

    # ---- per-sample constants + host tails ----
    tails = np.zeros(B)
    all_consts = np.zeros((B, NCONST))
    all_piv = []
    for b in range(B):
        A3, a4, piv, consts = _per_sample_host(tt[b], tr[b], te[b], re_[b])
        all_consts[b] = consts
        all_piv.append(piv)
        qtail = pc[b, :3, NDEV:].astype(np.float64)
        dtail = A3 @ qtail + a4[:, None]
        tails[b] = np.linalg.norm(dtail, axis=0).sum()

    # ---- per-core inputs (u1 prescaled by s1, shifted by b1) ----
    eye = np.eye(NPART, dtype=np.float16)
    in_maps = []
    for k in range(NCORES):
        pc_k = np.empty((3, NPART, FREE), np.float32)
        cst_k = np.empty((NPART, NCONST), np.float32)
        wg = np.zeros((NPART, NPART), np.float16)
        for j in range(SPC):
            b = k * SPC + j
            rows = slice(32 * j, 32 * (j + 1))
            s1, b1 = all_consts[b][3], all_consts[b][4]
            u1 = pc[b, all_piv[b][0], :NDEV] * np.float32(s1) + np.float32(b1)
            pc_k[0, rows, :] = u1.reshape(SLOTS, FREE)
            pc_k[1, rows, :] = pc[b, all_piv[b][1], :NDEV].reshape(SLOTS, FREE)
            pc_k[2, rows, :] = pc[b, all_piv[b][2], :NDEV].reshape(SLOTS, FREE)
            cst_row = all_consts[b].copy()
            cst_row[3], cst_row[4] = 1.0, 0.0   # folded into the u1 stream
            cst_k[rows, :] = cst_row
            gp = np.float16(all_consts[b][2])
            for kk in range(32 * j, 32 * (j + 1)):
                wg[kk, kk] = gp
        in_maps.append({
            "pc": pc_k,
            "cst": cst_k,
            "wts": np.concatenate([wg, eye], axis=1),
        })

    nc = _get_nc()
    res = run_bass_kernel_spmd(nc, in_maps, core_ids=list(range(NCORES)),
                               trace=trace)

    # ---- combine (host, float64) ----
    pcl_sum = 0.0
    for k in range(NCORES):
        acc = res.results[k]["acc"].astype(np.float64)
        for j in range(SPC):
            b = k * SPC + j
            pcl_sum += (acc[32 * j:32 * (j + 1), :].sum() + tails[b]) / N

    total = 0.5 * pose_loss + 0.5 * (pcl_sum / B)
    out = (np.float32(total), np.float32(loss_transl), np.float32(loss_rot),
           np.float32(pcl_sum / B))
    return out, res


def kernel(point_clouds, target_transl, target_rot, transl_err, rot_err):
    out, _ = _kernel_impl(point_clouds, target_transl, target_rot,
                          transl_err, rot_err)
    return out


# revision 25
# speedup vs baseline: 1.0530x; 1.0530x over previous
"""Trainium2 Bass kernel for nn_CombinedLoss (LCCNet CombinedLoss).

Per sample, displacement d = A3 @ q + a4. Column-pivoted QR gives
    err^2 = (s1*(u1 + a*u2 + b*u3) + b1)^2 + (s2*(u2 + g*u3) + b2)^2 + b3^2
with bounded |a|,|b|,|g| <= 1. Host prescales u1' = s1*u1 + b1 so the
first square needs no scale/bias (t1 = f1'^2, a plain DVE multiply).

Device mapping (8 cores, data-parallel; a core's 4 samples share the
partition dim: p = 32*sample + slot, each slot FREE=6240 points):
  - Pool: chunked fp32->fp16 cast DMAs (the ~25us read roofline) plus
    the three big s12 = t1+t2 adds.
  - PE: c2 = g.*u3 + u2 via two diagonal-matmul accumulates per 390-col
    PSUM bank slot (slot table avoids bank wraps); ACT reads PSUM.
  - DVE: c1' = (b*s1).*u3 + u1', f1' = (a*s1).*u2 + c1' (STT), the small
    s12 adds, and t1 = f1'*f1' for the two middle chunks.
  - ACT: t2 = Square(s2.*c2_psum + b2), t1 squares for the small chunks,
    err = Sqrt(s12 + b3sq) with accum_out (sqrts merged pairwise at the
    edges to cut accumulator reads).
  - Chunks [390,390,1560,1560,1560,390,390]: fast fill, short tail.
  - Host: tails + pose loss in float64.
"""

import numpy as np

B = 32
N = 200000
NCORES = 8
SPC = B // NCORES
NPART = 128
SLOTS = 32
FREE = 6240
NDEV = SLOTS * FREE        # 199680 points per sample on device
NCONST = 8

DMA_CH = [780] * 7 + [390, 390]
CCH = [780, 780, 1560, 1560, 780, 390, 390]
PEW = 390
NPAIR = FREE // PEW        # 16
NSLOT = 8
SLOT_STRIDE = 512          # fp32 elems per 2KB PSUM bank

# PSUM bank slot per PE pair (chunk's pairs stay contiguous, no wraps)
PAIR_SLOT = [p % NSLOT for p in range(NPAIR)]

_CACHED_NC = None


def _offsets(widths):
    offs, o = [], 0
    for w in widths:
        offs.append(o)
        o += w
    return offs


DMA_OFF = _offsets(DMA_CH)
C_OFF = _offsets(CCH)
C_DMA = [[d for d in range(len(DMA_CH))
          if DMA_OFF[d] < o + w and DMA_OFF[d] + DMA_CH[d] > o]
         for o, w in zip(C_OFF, CCH)]
PAIR_DMA = [next(d for d in range(len(DMA_CH))
                 if DMA_OFF[d] <= p * PEW < DMA_OFF[d] + DMA_CH[d])
            for p in range(NPAIR)]
C_PAIR = [(o // PEW, w // PEW) for o, w in zip(C_OFF, CCH)]

DVE_T1 = ()                # all t1 squares on ACT (v2 balance)
SQ_OPS = [((c,), o, w, c) for c, (o, w) in enumerate(zip(C_OFF, CCH))]
NACC = len(SQ_OPS)


def _quat_to_rot(q):
    q = q / np.linalg.norm(q)
    w, x, y, z = q
    return np.array([
        [1 - 2*y*y - 2*z*z, 2*x*y - 2*z*w,     2*x*z + 2*y*w],
        [2*x*y + 2*z*w,     1 - 2*x*x - 2*z*z, 2*y*z - 2*x*w],
        [2*x*z - 2*y*w,     2*y*z + 2*x*w,     1 - 2*x*x - 2*y*y],
    ])


def _pivoted_qr(A3):
    """Column-pivoted QR (float64): A3[:, piv] = Q @ R, |R[i,j]| <= |R[i,i]|."""
    cols = {c: A3[:, c].astype(np.float64).copy() for c in range(3)}
    coeff = {c: np.zeros(3) for c in range(3)}
    remaining = [0, 1, 2]
    piv = []
    Q = np.zeros((3, 3))
    for i in range(3):
        cbest = max(remaining, key=lambda c: float(np.dot(cols[c], cols[c])))
        remaining.remove(cbest)
        piv.append(cbest)
        v = cols[cbest]
        nrm = np.sqrt(np.dot(v, v))
        if nrm < 1e-300:
            for basis in np.eye(3):
                w = basis - Q[:, :i] @ (Q[:, :i].T @ basis)
                if np.dot(w, w) > 1e-12:
                    v = w
                    break
            nrm = np.sqrt(np.dot(v, v))
        q = v / nrm
        Q[:, i] = q
        for c in [cbest] + remaining:
            proj = float(np.dot(q, cols[c]))
            coeff[c][i] = proj
            cols[c] = cols[c] - proj * q
    R = np.stack([coeff[c] for c in piv], axis=1)
    return Q, R, piv


def _per_sample_host(tt, tr, te, re_):
    """(A3, a4, piv, consts). consts = [a*s1, b*s1, g, s1, b1, s2, b2, b3sq]
    where s1=r11, b1=b4[0] also get folded into the u1 stream on the host."""
    R_t = _quat_to_rot(tr.astype(np.float64))
    R_p = _quat_to_rot(re_.astype(np.float64))
    A3 = R_p.T @ R_t - np.eye(3)
    a4 = R_p.T @ (tt.astype(np.float64) - te.astype(np.float64))
    Q, R, piv = _pivoted_qr(A3)
    b4 = Q.T @ a4
    r11, r12, r13 = R[0, 0], R[0, 1], R[0, 2]
    r22, r23 = R[1, 1], R[1, 2]
    alpha = r12 / r11 if abs(r11) > 1e-30 else 0.0
    beta = r13 / r11 if abs(r11) > 1e-30 else 0.0
    gamma = r23 / r22 if abs(r22) > 1e-30 else 0.0
    consts = np.array([alpha * r11, beta * r11, gamma, r11, b4[0], r22,
                       b4[1], b4[2] ** 2])
    return A3, a4, piv, consts


def _build_nc():
    import concourse.bass as bass
    from concourse import mybir
    from contextlib import ExitStack

    f16, f32 = mybir.dt.float16, mybir.dt.float32
    Alu = mybir.AluOpType
    Act = mybir.ActivationFunctionType

    nc = bass.Bass("TRN2", target_bir_lowering=False, debug=False,
                   num_devices=NCORES)
    pc = nc.dram_tensor("pc", [3, NPART, FREE], f32, kind="ExternalInput").ap()
    cst = nc.dram_tensor("cst", [NPART, NCONST], f32,
                         kind="ExternalInput").ap()
    wts = nc.dram_tensor("wts", [NPART, 2 * NPART], f16,
                         kind="ExternalInput").ap()
    acc_out = nc.dram_tensor("acc", [NPART, NACC], f32,
                             kind="ExternalOutput").ap()

    NCH = len(CCH)

    # ---- ACT program order and tick numbering (1 inc per ACT op) ----
    act_order = [("t2", 0), ("t1", 0), ("t2", 1), ("t1", 1), ("sq", 0)]
    for _c in range(2, len(CCH)):
        act_order += [("t2", _c), ("t1", _c), ("sq", _c - 1)]
    act_order.append(("sq", len(CCH) - 1))
    ACT_TICK = {op: i + 1 for i, op in enumerate(act_order)}
    ACT_TOTAL = len(act_order)

    # ---- DVE sem_dve tick numbering ----
    dve_order = [("f1", _c) for _c in range(len(CCH))]
    DVE_TICK = {op: i + 1 for i, op in enumerate(dve_order)}
    # s12 runs on DVE in program order; sem_s12 count per chunk
    s12_order = list(range(len(CCH)))
    S12_TICK = {c: i + 1 for i, c in enumerate(s12_order)}

    # PE slot-reuse waits: pair -> required sem_act tick (slot written 8
    # pairs earlier must have been consumed by that chunk's t2)
    PE_WAIT = {8: ACT_TICK[("t2", 0)], 10: ACT_TICK[("t2", 1)],
               12: ACT_TICK[("t2", 2)]}

    with ExitStack() as ctx:
        E = ctx.enter_context
        u = E(nc.sbuf_tensor("u", [NPART, 3 * FREE], f16))
        ct = E(nc.sbuf_tensor("ct", [NPART, NCONST], f32))
        wt = E(nc.sbuf_tensor("wt", [NPART, 2 * NPART], f16))
        c1b = E(nc.sbuf_tensor("c1b", [NPART, FREE], f16))
        f1b = E(nc.sbuf_tensor("f1b", [NPART, FREE], f16))
        t1b = E(nc.sbuf_tensor("t1b", [NPART, FREE], f16))
        t2b = E(nc.sbuf_tensor("t2b", [NPART, FREE], f16))
        s12b = E(nc.sbuf_tensor("s12b", [NPART, FREE], f16))
        esb = E(nc.sbuf_tensor("esb", [NPART, 1560], f16))
        acc = E(nc.sbuf_tensor("acc_sb", [NPART, NACC], f32))
        ps = E(nc.psum_tensor("ps", [NPART, NSLOT * SLOT_STRIDE], f32))

        sem_c = E(nc.semaphore("sem_c"))
        sem_u = [E(nc.semaphore(f"sem_u{d}")) for d in range(len(DMA_CH))]
        sem_pe = E(nc.semaphore("sem_pe"))
        sem_dve = E(nc.semaphore("sem_dve"))
        sem_act = E(nc.semaphore("sem_act"))
        sem_s12 = E(nc.semaphore("sem_s12"))
        sem_out = E(nc.semaphore("sem_out"))
        block = E(nc.Block())

        def ccol(i):
            return ct[:, i:i + 1]

        def useg(coord, o, w):
            return u[:, coord * FREE + o:coord * FREE + o + w]

        wg = wt[:, 0:NPART]
        wi = wt[:, NPART:2 * NPART]

        @block.gpsimd
        def _(g):
            def chunk_dma(d):
                o, w = DMA_OFF[d], DMA_CH[d]
                g.dma_start(
                    u[:].rearrange("p (c f) -> p c f", c=3)[:, :, o:o + w],
                    pc[:, :, o:o + w].rearrange("c p f -> p c f"),
                ).then_inc(sem_u[d], 16)

            chunk_dma(0)
            g.dma_start(ct[:], cst).then_inc(sem_c, 16)
            g.dma_start(wt[:], wts).then_inc(sem_c, 16)
            for d in range(1, len(DMA_CH)):
                chunk_dma(d)

        @block.tensor
        def _(t):
            t.wait_ge(sem_c, 32)
            for _i in range(6):
                t.matmul(ps[:, 7 * SLOT_STRIDE:7 * SLOT_STRIDE + PEW],
                         wi, useg(2, 0, PEW), start=True, stop=True)
            last_d = -1
            for p in range(NPAIR):
                d = PAIR_DMA[p]
                if d != last_d:
                    t.wait_ge(sem_u[d], 16)
                    last_d = d
                if p in PE_WAIT:
                    t.wait_ge(sem_act, PE_WAIT[p])
                s0 = PAIR_SLOT[p] * SLOT_STRIDE
                slot = ps[:, s0:s0 + PEW]
                o = p * PEW
                t.matmul(slot, wg, useg(2, o, PEW), start=True, stop=False)
                t.matmul(slot, wi, useg(1, o, PEW), start=False, stop=True) \
                    .then_inc(sem_pe, 1)

        @block.vector
        def _(v):
            v.wait_ge(sem_c, 16)

            def K(c):
                o, w = C_OFF[c], CCH[c]
                for d in C_DMA[c]:
                    v.wait_ge(sem_u[d], 16)
                v.scalar_tensor_tensor(c1b[:, o:o + w], useg(2, o, w),
                                       ccol(1), useg(0, o, w),
                                       Alu.mult, Alu.add)
                v.drain()  # f1 reads c1 on the same engine
                v.scalar_tensor_tensor(f1b[:, o:o + w], useg(1, o, w),
                                       ccol(0), c1b[:, o:o + w],
                                       Alu.mult, Alu.add) \
                    .then_inc(sem_dve, 1)

            def t1d(c):
                o, w = C_OFF[c], CCH[c]
                v.drain()  # reads f1 written by the previous STT
                v.tensor_tensor(t1b[:, o:o + w], f1b[:, o:o + w],
                                f1b[:, o:o + w], Alu.mult) \
                    .then_inc(sem_dve, 1)

            def S(c):
                o, w = C_OFF[c], CCH[c]
                if c in DVE_T1:
                    # t1 is local; only t2 comes from ACT
                    v.wait_ge(sem_act, ACT_TICK[("t2", c)])
                else:
                    v.wait_ge(sem_act, ACT_TICK[("t1", c)])
                v.tensor_tensor(s12b[:, o:o + w], t1b[:, o:o + w],
                                t2b[:, o:o + w], Alu.add).then_inc(sem_s12, 1)

            K(0)
            K(1)
            S(0)
            for _c in range(2, len(CCH)):
                K(_c)
                S(_c - 1)
            S(len(CCH) - 1)

        @block.scalar
        def _(s):
            # prime the Square/Sqrt table off the critical path
            s.activation(esb[:, 0:1], esb[:, 1:2], Act.Square)
            s.wait_ge(sem_c, 16)
            sq_done = [0]

            def t2(c):
                o, w = C_OFF[c], CCH[c]
                p0, np_ = C_PAIR[c]
                s.wait_ge(sem_pe, p0 + np_)
                s0 = PAIR_SLOT[p0]
                pin = ps[:].rearrange("p (k q) -> p k q", k=NSLOT)[
                    :, s0:s0 + np_, 0:PEW]
                s.activation(
                    t2b[:, o:o + w].rearrange("p (k f) -> p k f", k=np_),
                    pin, Act.Square, bias=ccol(6), scale=ccol(5)) \
                    .then_inc(sem_act, 1)

            def t1(c):
                o, w = C_OFF[c], CCH[c]
                s.wait_ge(sem_dve, DVE_TICK[("f1", c)])
                s.activation(t1b[:, o:o + w], f1b[:, o:o + w], Act.Square,
                             bias=ccol(4), scale=ccol(3)) \
                    .then_inc(sem_act, 1)

            def sq(i):
                chunks, o, w, col = SQ_OPS[i]
                s.wait_ge(sem_s12, S12_TICK[chunks[-1]])
                s.activation(esb[:, 0:w], s12b[:, o:o + w], Act.Sqrt,
                             bias=ccol(7), accum_out=acc[:, col:col + 1]) \
                    .then_inc(sem_act, 1)

            emit = {"t2": t2, "t1": t1}
            for kind, cc in act_order:
                if kind == "sq":
                    sq(cc)
                else:
                    emit[kind](cc)

        @block.sync
        def _(sp):
            sp.wait_ge(sem_act, ACT_TOTAL)
            sp.dma_start(acc_out, acc[:]).then_inc(sem_out, 16)
            sp.wait_ge(sem_out, 16)

    return nc


def _get_nc():
    global _CACHED_NC
    if _CACHED_NC is None:
        _CACHED_NC = _build_nc()
    return _CACHED_NC


def _kernel_impl(point_clouds, target_transl, target_rot, transl_err, rot_err,
                 trace=False):
    from concourse.bass_utils import run_bass_kernel_spmd

    pc = np.asarray(point_clouds)
    tt = np.asarray(target_transl, np.float64)
    tr = np.asarray(target_rot, np.float64)
    te = np.asarray(transl_err, np.float64)
    re_ = np.asarray(rot_err, np.float64)

    # ---- pose loss (host, float64) ----
    d = np.abs(te - tt)
    loss_transl = np.where(d < 1.0, 0.5 * d * d, d - 0.5).sum(axis=1).mean()

    rinv = tr * np.array([1.0, -1.0, -1.0, -1.0])
    q = re_
    w = q[:, 0]*rinv[:, 0] - q[:, 1]*rinv[:, 1] - q[:, 2]*rinv[:, 2] - q[:, 3]*rinv[:, 3]
    x = q[:, 0]*rinv[:, 1] + q[:, 1]*rinv[:, 0] + q[:, 2]*rinv[:, 3] - q[:, 3]*rinv[:, 2]
    y = q[:, 0]*rinv[:, 2] - q[:, 1]*rinv[:, 3] + q[:, 2]*rinv[:, 0] + q[:, 3]*rinv[:, 1]
    z = q[:, 0]*rinv[:, 3] + q[:, 1]*rinv[:, 2] - q[:, 2]*rinv[:, 1] + q[:, 3]*rinv[:, 0]
    angle = 2.0 * np.arctan2(np.sqrt(x*x + y*y + z*z), np.abs(w))
    loss_rot = (180.0 * angle / np.pi).mean()
    pose_loss = loss_transl + loss_rot

    # ---- per-sample constants + host tails ----
    tails = np.zeros(B)
    all_consts = np.zeros((B, NCONST))
    all_piv = []
    for b in range(B):
        A3, a4, piv, consts = _per_sample_host(tt[b], tr[b], te[b], re_[b])
        all_consts[b] = consts
        all_piv.append(piv)
        qtail = pc[b, :3, NDEV:].astype(np.float64)
        dtail = A3 @ qtail + a4[:, None]
        tails[b] = np.linalg.norm(dtail, axis=0).sum()

    # ---- per-core inputs (u1 prescaled by s1, shifted by b1) ----
    eye = np.eye(NPART, dtype=np.float16)
    in_maps = []
    for k in range(NCORES):
        pc_k = np.empty((3, NPART, FREE), np.float32)
        cst_k = np.empty((NPART, NCONST), np.float32)
        wg = np.zeros((NPART, NPART), np.float16)
        for j in range(SPC):
            b = k * SPC + j
            rows = slice(32 * j, 32 * (j + 1))
            s1, b1 = all_consts[b][3], all_consts[b][4]
            u1 = pc[b, all_piv[b][0], :NDEV] * np.float32(s1) + np.float32(b1)
            pc_k[0, rows, :] = u1.reshape(SLOTS, FREE)
            pc_k[1, rows, :] = pc[b, all_piv[b][1], :NDEV].reshape(SLOTS, FREE)
            pc_k[2, rows, :] = pc[b, all_piv[b][2], :NDEV].reshape(SLOTS, FREE)
            cst_row = all_consts[b].copy()
            cst_row[3], cst_row[4] = 1.0, 0.0   # folded into the u1 stream
            cst_k[rows, :] = cst_row
            gp = np.float16(all_consts[b][2])
            for kk in range(32 * j, 32 * (j + 1)):
                wg[kk, kk] = gp
        in_maps.append({
            "pc": pc_k,
            "cst": cst_k,
            "wts": np.concatenate([wg, eye], axis=1),
        })

    nc = _get_nc()
    res = run_bass_kernel_spmd(nc, in_maps, core_ids=list(range(NCORES)),
                               trace=trace)

    # ---- combine (host, float64) ----
    pcl_sum = 0.0
    for k in range(NCORES):
        acc = res.results[k]["acc"].astype(np.float64)
        for j in range(SPC):
            b = k * SPC + j
            pcl_sum += (acc[32 * j:32 * (j + 1), :].sum() + tails[b]) / N

    total = 0.5 * pose_loss + 0.5 * (pcl_sum / B)
    out = (np.float32(total), np.float32(loss_transl), np.float32(loss_rot),
           np.float32(pcl_sum / B))
    return out, res


def kernel(point_clouds, target_transl, target_rot, transl_err, rot_err):
    out, _ = _kernel_impl(point_clouds, target_transl, target_rot,
                          transl_err, rot_err)
    return out
